# Initial kernel scaffold
#
"""Trainium2 Bass kernel for linear (taylor/sparse) attention.

Reference computation (per batch b, with xf = x.reshape(b, C, N)):
    Q = Wq@xf + bq            [Cqk, N]
    K = Wk@xf + bk            [Cqk, N]
    V = Wv@xf + bv            [C, N]
    Qh = Q / ||Q||_2 (per position, channel dim)
    Kh = K / ||K||_2
    tailor[n]   = 1 / (N + Qh[:,n] . (sum_n Kh + eps))
    matrix      = Kh @ V^T    [Cqk, C]
    out[:, n]   = gamma * tailor[n] * (sum_n V + matrix^T @ Qh[:,n])

Distribution: 8 cores = 4 batches x 2 halves of N. Each core computes the
local factor F = [Kh_aug @ [V | 1]] in one accumulated PSUM tile:
    F[0:32, 0:256]  = Kh @ V'^T   (V' = gamma*Wv@x, bias folded in later)
    F[0:32, 256]    = sum Kh
    F[32,   0:256]  = sum V'
    F[32,   256]    = N_local
then a pairwise AllReduce (34 KB) makes F global, and phase 2 computes the
output via one GEMM per 128-position tile:
    psum2[n, 0:256] = Q_aug^T @ Mx  (Q_aug rows 0-31 = biased Q, row 32 = ||Q||)
    psum2[n, 256]   = denominator (N*||Q|| + Q . (Ksum+eps)) via extra Mx column
    out^T[n, :]     = psum2[n, 0:256] / psum2[n, 256]
gamma is folded into Wv/bv on the host; the V bias is folded into the factors
after the AllReduce (value_sum += N*bv', matrix += Ksum (x) bv'). The Q bias
enters the norm via ||Q+bq||^2 = ||Qraw||^2 + 2*bq.Qraw + ||bq||^2, with
bq.Qraw computed by an extra (Wq^T bq) column of the fused projection.

Output is written n-major ([N_shard, C]); the host transposes back.
"""

import ml_dtypes
import numpy as np
from contextlib import ExitStack

import concourse.bass as bass
import concourse.bacc as bacc
import concourse.tile as tile
from concourse import mybir
from concourse import bass_utils
from concourse.masks import make_identity

F32 = mybir.dt.float32
BF16 = mybir.dt.bfloat16
ALU = mybir.AluOpType
ACTF = mybir.ActivationFunctionType

B, C, HH, WW = 4, 256, 128, 128
N = HH * WW            # 16384 positions per batch
NSH = N // 2           # 8192 positions per core
CQK = 32
WID = 2 * CQK + C      # 320: [Q | K | V] fused projection width
KVW = WID + 2          # 322: kvres = [Q+bq | K+bk | V | ones ones]
FD = C + 2             # 258: factor / Mx / psum2 free width
NT512 = NSH // 512     # 16
NT128 = NSH // 128     # 64
GRP = 8                # tiles per norm-batching group
EPS = 1e-6

_CACHE = {}


def _build():
    nc = bacc.Bacc("TRN2", target_bir_lowering=False, debug=False, num_devices=8)

    xs = nc.dram_tensor("xs", [C, NSH], BF16, kind="ExternalInput").ap()
    wcat = nc.dram_tensor("wcat", [C, WID], BF16, kind="ExternalInput").ap()
    biaskv = nc.dram_tensor("biaskv", [WID], F32, kind="ExternalInput").ap()
    bq_in = nc.dram_tensor("bq", [CQK, 1], F32, kind="ExternalInput").ap()
    bvg = nc.dram_tensor("bvg", [C], F32, kind="ExternalInput").ap()
    out = nc.dram_tensor("out", [NSH, C], F32, kind="ExternalOutput").ap()

    with tile.TileContext(nc) as tc, ExitStack() as ctx:
        _body(ctx, tc, nc, xs, wcat, biaskv, bq_in, bvg, out)

    nc.compile()
    return nc


def _body(ctx, tc, nc, xs, wcat, biaskv, bq_in, bvg, out):
    singles = ctx.enter_context(tc.tile_pool(name="singles", bufs=1))
    xpool = ctx.enter_context(tc.tile_pool(name="x", bufs=NT512))
    kvpool = ctx.enter_context(tc.tile_pool(name="kv", bufs=2 * GRP))
    khpool = ctx.enter_context(tc.tile_pool(name="kh", bufs=4))
    smalls = ctx.enter_context(tc.tile_pool(name="smalls", bufs=4))
    scpool = ctx.enter_context(tc.tile_pool(name="scratch", bufs=4))
    outpool = ctx.enter_context(tc.tile_pool(name="outp", bufs=3))

    ps_sh = ctx.enter_context(tc.tile_pool(name="ps_sh", bufs=4, space="PSUM"))
    ps_kqv = ctx.enter_context(tc.tile_pool(name="ps_kqv", bufs=3, space="PSUM"))
    ps_f = ctx.enter_context(tc.tile_pool(name="ps_f", bufs=1, space="PSUM"))
    dram = ctx.enter_context(tc.tile_pool(name="dram", bufs=1, space="DRAM"))

    # ---- one-time setup ----
    wcat_sb = singles.tile([128, 2, WID], BF16)
    nc.sync.dma_start(wcat_sb[:], wcat.rearrange("(cb cp) w -> cp cb w", cb=2))
    biaskv_rep = singles.tile([128, WID], F32)  # [bq | bk | zeros(C)]
    nc.gpsimd.dma_start(
        biaskv_rep[:], biaskv.unsqueeze(0).partition_broadcast(128).squeeze(1)
    )
    bq_col = singles.tile([CQK, 1], F32)
    nc.gpsimd.dma_start(bq_col[:], bq_in)
    bv_rep = singles.tile([CQK + 1, C], F32)
    nc.gpsimd.dma_start(
        bv_rep[:], bvg.unsqueeze(0).partition_broadcast(CQK + 1).squeeze(1)
    )
    ident = singles.tile([128, 128], F32)
    make_identity(nc, ident[:])
    ones2 = singles.tile([128, 2], F32)
    nc.vector.memset(ones2[:], 1.0)

    qx = singles.tile([CQK + 1, NSH], BF16)         # layout-A Q rows + ||Q|| row
    ssq_stack = singles.tile([128, NT128], F32)     # sum((Q+bq)^2), col t
    ssk_stack = singles.tile([128, NT128], F32)     # sum((K+bk)^2), col t
    rnormk_stack = singles.tile([128, NT128], F32)
    psf = ps_f.tile([CQK + 1, FD], F32)             # factor accumulator

    kvres_tiles = [None] * NT128
    xt_tiles = [None] * NT512
    pending_tail = None

    def emit_tail(g0):
        normk_g = smalls.tile([128, GRP], F32)
        nc.scalar.sqrt(normk_g[:], ssk_stack[:, g0 : g0 + GRP])
        nc.vector.reciprocal(rnormk_stack[:, g0 : g0 + GRP], normk_g[:])
        for tt in range(g0, g0 + GRP):
            kvt = kvres_tiles[tt]
            kh = khpool.tile([128, CQK + 1], BF16)
            if tt % 2 == 0:
                nc.vector.tensor_scalar_mul(
                    kh[:, 0:CQK], kvt[:, CQK : 2 * CQK], rnormk_stack[:, tt : tt + 1]
                )
            else:
                nc.scalar.mul(
                    kh[:, 0:CQK], kvt[:, CQK : 2 * CQK],
                    rnormk_stack[:, tt : tt + 1],
                )
            nc.gpsimd.tensor_copy(kh[:, CQK : CQK + 1], ones2[:, 0:1])
            nc.tensor.matmul(
                psf[:], kh[:], kvt[:, 2 * CQK : KVW],
                start=(tt == 0), stop=(tt == NT128 - 1),
            )

    # ---- phase 1 ----
    for j in range(NT512):
        xt = xpool.tile([128, 2, 512], BF16)
        nc.sync.dma_start(
            xt[:],
            xs.rearrange("(cb cp) n -> cp cb n", cb=2)[:, :, j * 512 : (j + 1) * 512],
        )

        xt_tiles[j] = xt

        for u in range(4):
            t = j * 4 + u
            if u == 2 and j % 2 == 0 and pending_tail is not None:
                emit_tail(pending_tail)
                pending_tail = None
            # fused [Q^T | K^T | V^T] projection, n-major: [128, 320]
            pskqv = ps_kqv.tile([128, WID], F32)
            for cb in range(2):
                nc.tensor.matmul(
                    pskqv[:], xt[:, cb, u * 128 : (u + 1) * 128], wcat_sb[:, cb, :],
                    start=(cb == 0), stop=(cb == 1),
                )
            # kvres = [Q+bq | K+bk | V | (junk -> ones)]
            kv = kvpool.tile([128, KVW], BF16)
            kvres_tiles[t] = kv
            nc.vector.tensor_tensor(
                kv[:, 0:WID], pskqv[:], biaskv_rep[:], ALU.add
            )
            nc.gpsimd.tensor_copy(kv[:, WID:KVW], ones2[:])
            scr_q = scpool.tile([128, CQK], BF16)
            scr_k = scpool.tile([128, CQK], BF16)
            if t % 2 == 0:
                nc.scalar.activation(
                    scr_q[:], kv[:, 0:CQK], ACTF.Square,
                    accum_out=ssq_stack[:, t : t + 1],
                )
                nc.vector.scalar_tensor_tensor(
                    scr_k[:], kv[:, CQK : 2 * CQK], 1.0, kv[:, CQK : 2 * CQK],
                    ALU.mult, ALU.mult, accum_out=ssk_stack[:, t : t + 1],
                )
            else:
                nc.vector.scalar_tensor_tensor(
                    scr_q[:], kv[:, 0:CQK], 1.0, kv[:, 0:CQK],
                    ALU.mult, ALU.mult, accum_out=ssq_stack[:, t : t + 1],
                )
                nc.scalar.activation(
                    scr_k[:], kv[:, CQK : 2 * CQK], ACTF.Square,
                    accum_out=ssk_stack[:, t : t + 1],
                )

        # ---- group tail (deferred): batched K-norms + factor matmuls ----
        if (j + 1) % (GRP // 4) == 0:
            pending_tail = (j + 1) * 4 - GRP
    if pending_tail is not None:
        emit_tail(pending_tail)
        pending_tail = None

    # ---- phase 1.5: ||Q|| row + AllReduce of factors ----
    normq_stack = singles.tile([128, NT128], F32)
    nc.scalar.sqrt(normq_stack[:], ssq_stack[:])
    pst = ps_sh.tile([NT128, 128], F32, tag="shared")
    nc.tensor.transpose(pst[:], normq_stack[:], ident[:])
    trT = singles.tile([NT128, 128], BF16)
    nc.vector.tensor_copy(trT[:], pst[:])
    row_scratch = dram.tile([NT128, 128], BF16)
    nc.sync.dma_start(row_scratch[:], trT[:])
    nc.sync.dma_start(
        qx[CQK : CQK + 1, :],
        row_scratch[:].rearrange("a b -> (a b)").unsqueeze(0),
    )

    fac_loc = singles.tile([CQK + 1, FD], F32)
    nc.vector.tensor_copy(fac_loc[:], psf[:])
    cc_in = dram.tile([CQK + 1, FD], F32)
    cc_out = dram.tile([2 * (CQK + 1), FD], F32)
    nc.sync.dma_start(cc_in[:], fac_loc[:])
    nc.gpsimd.collective_compute(
        "AllGather",
        ALU.bypass,
        replica_groups=[[0, 1], [2, 3], [4, 5], [6, 7]],
        ins=[cc_in.opt()],
        outs=[cc_out.opt()],
    )
    fac2 = singles.tile([CQK + 1, 2, FD], F32)
    nc.sync.dma_start(fac2[:], cc_out[:].rearrange("(r p) f -> p r f", r=2))
    # ---- gap work: layout-A Q tiles + qx rows (only needed by phase 2) ----
    for j in range(NT512):
        psq = ps_sh.tile([CQK, 512], F32, tag="shared")
        for cb in range(2):
            nc.tensor.matmul(
                psq[:], wcat_sb[:, cb, 0:CQK], xt_tiles[j][:, cb, :],
                start=(cb == 0), stop=(cb == 1),
            )
        nc.scalar.activation(
            qx[0:CQK, j * 512 : (j + 1) * 512], psq[:],
            ACTF.Identity, bias=bq_col[:], scale=1.0,
        )

    # PE warm-keeper (independent of the collective): DVE delay ladder with a
    # dummy matmul after each rung so HAM stays at full clock across the gap.
    warm_a = singles.tile([128, 4096], F32)
    warm_b = singles.tile([128, 4096], F32)
    nc.vector.memset(warm_a[:], 1.0)
    for w in range(8):
        src_t, dst_t = (warm_a, warm_b) if w % 2 == 0 else (warm_b, warm_a)
        nc.vector.tensor_copy(dst_t[:], src_t[:])
        pw = ps_kqv.tile([128, 256], F32, tag="pskqv")
        nc.tensor.matmul(
            pw[:], dst_t[:, 0:128], dst_t[:, 0:256], start=True, stop=True
        )

    facg = singles.tile([CQK + 1, FD], F32)
    nc.vector.tensor_tensor(facg[:], fac2[:, 0, :], fac2[:, 1, :], ALU.add)

    # ---- build Mx [33, 258]:
    #   rows 0-31, cols 0-255: matrix' = facg + Ksum (x) bv'
    #   row 32,    cols 0-255: value_sum' = facg_row32 + N * bv'
    #   col 256:   [Ksum + eps ; N]  (denominator column); col 257 pad
    mx = singles.tile([CQK + 1, FD], BF16)
    tmp32 = singles.tile([CQK, C], F32)
    nc.vector.tensor_scalar_mul(tmp32[:], bv_rep[0:CQK, :], facg[0:CQK, C : C + 1])
    nc.vector.tensor_tensor(mx[0:CQK, 0:C], tmp32[:], facg[0:CQK, 0:C], ALU.add)
    nc.vector.scalar_tensor_tensor(
        mx[CQK : CQK + 1, 0:C], bv_rep[CQK : CQK + 1, :], float(N),
        facg[CQK : CQK + 1, 0:C],
        ALU.mult, ALU.add,
    )
    nc.vector.tensor_scalar_add(mx[0 : CQK + 1, C:FD], facg[0 : CQK + 1, C:FD], EPS)

    # ---- phase 2 ----
    out4 = out.rearrange("(t4 u p) c -> t4 p u c", u=4, p=128)
    for t4 in range(NT128 // 4):
        ot = outpool.tile([128, 4, C], F32)
        for u in range(4):
            t = t4 * 4 + u
            ps2 = ps_sh.tile([128, FD], F32, tag="shared")
            nc.tensor.matmul(
                ps2[:], qx[:, t * 128 : (t + 1) * 128], mx[:], start=True, stop=True
            )
            s_col = smalls.tile([128, 1], F32)
            nc.vector.reciprocal(s_col[:], ps2[:, C : C + 1])
            if t % 2 == 0:
                nc.vector.tensor_scalar_mul(ot[:, u, :], ps2[:, 0:C], s_col[:])
            else:
                nc.scalar.mul(ot[:, u, :], ps2[:, 0:C], s_col[:])
        nc.sync.dma_start(out4[t4], ot[:])


def _get_nc():
    if "nc" not in _CACHE:
        _CACHE["nc"] = _build()
    return _CACHE["nc"]


def _prep_in_maps(x, Wq, bq, Wk, bk, Wv, bv, gamma):
    g = float(np.asarray(gamma).reshape(-1)[0])
    wcat = np.concatenate(
        [
            Wq.T.astype(np.float32),
            Wk.T.astype(np.float32),
            (g * Wv).T.astype(np.float32),
        ],
        axis=1,
    ).astype(ml_dtypes.bfloat16)
    wcat = np.ascontiguousarray(wcat)
    biaskv = np.concatenate(
        [bq.astype(np.float32), bk.astype(np.float32), np.zeros(C, np.float32)]
    )
    bvg = np.ascontiguousarray(g * bv, dtype=np.float32)
    bq_col = np.ascontiguousarray(bq.reshape(CQK, 1), dtype=np.float32)

    xf = np.asarray(x, dtype=np.float32).reshape(B, C, N)
    in_maps = []
    for core in range(8):
        b, h = core // 2, core % 2
        xsh = np.ascontiguousarray(
            xf[b, :, h * NSH : (h + 1) * NSH].astype(ml_dtypes.bfloat16)
        )
        in_maps.append(
            {
                "xs": xsh,
                "wcat": wcat,
                "biaskv": biaskv,
                "bq": bq_col,
                "bvg": bvg,
            }
        )
    return in_maps


def run(inputs, trace=False):
    nc = _get_nc()
    in_maps = _prep_in_maps(**inputs)
    res = bass_utils.run_bass_kernel_spmd(
        nc, in_maps, core_ids=list(range(8)), trace=trace
    )
    outf = np.empty((B, C, N), np.float32)
    for core in range(8):
        b, h = core // 2, core % 2
        outf[b, :, h * NSH : (h + 1) * NSH] = res.results[core]["out"].T
    return outf.reshape(B, C, HH, WW), res


def kernel(**inputs):
    out, _ = run(inputs, trace=False)
    return out



# revision 1
# speedup vs baseline: 1.2612x; 1.2612x over previous
"""Trainium2 Bass kernel for linear (taylor/sparse) attention.

Reference computation (per batch b, with xf = x.reshape(b, C, N)):
    Q = Wq@xf + bq            [Cqk, N]
    K = Wk@xf + bk            [Cqk, N]
    V = Wv@xf + bv            [C, N]
    Qh = Q / ||Q||_2 (per position, channel dim)
    Kh = K / ||K||_2
    tailor[n]   = 1 / (N + Qh[:,n] . (sum_n Kh + eps))
    matrix      = Kh @ V^T    [Cqk, C]
    out[:, n]   = gamma * tailor[n] * (sum_n V + matrix^T @ Qh[:,n])

Distribution: 8 cores = 4 batches x 2 halves of N. Each core computes the
local factor F = [Kh_aug @ [V | 1]] in one accumulated PSUM tile:
    F[0:32, 0:256]  = Kh @ V'^T   (V' = gamma*Wv@x, bias folded in later)
    F[0:32, 256]    = sum Kh
    F[32,   0:256]  = sum V'
    F[32,   256]    = N_local
then a pairwise AllReduce (34 KB) makes F global, and phase 2 computes the
output via one GEMM per 128-position tile:
    psum2[n, 0:256] = Q_aug^T @ Mx  (Q_aug rows 0-31 = biased Q, row 32 = ||Q||)
    psum2[n, 256]   = denominator (N*||Q|| + Q . (Ksum+eps)) via extra Mx column
    out^T[n, :]     = psum2[n, 0:256] / psum2[n, 256]
gamma is folded into Wv/bv on the host; the V bias is folded into the factors
after the AllReduce (value_sum += N*bv', matrix += Ksum (x) bv'). The Q bias
enters the norm via ||Q+bq||^2 = ||Qraw||^2 + 2*bq.Qraw + ||bq||^2, with
bq.Qraw computed by an extra (Wq^T bq) column of the fused projection.

Output is written n-major ([N_shard, C]); the host transposes back.
"""

import ml_dtypes
import numpy as np
from contextlib import ExitStack

import concourse.bass as bass
import concourse.bacc as bacc
import concourse.tile as tile
from concourse import mybir
from concourse import bass_utils
from concourse.masks import make_identity

F32 = mybir.dt.float32
BF16 = mybir.dt.bfloat16
ALU = mybir.AluOpType
ACTF = mybir.ActivationFunctionType

B, C, HH, WW = 4, 256, 128, 128
N = HH * WW            # 16384 positions per batch
NSH = N // 2           # 8192 positions per core
CQK = 32
WID = 2 * CQK + C      # 320: [Q | K | V] fused projection width
KVW = WID + 2          # 322: kvres = [Q+bq | K+bk | V | ones ones]
FD = C + 2             # 258: factor / Mx / psum2 free width
NT512 = NSH // 512     # 16
NT128 = NSH // 128     # 64
GRP = 8                # tiles per norm-batching group
EPS = 1e-6

_CACHE = {}


def _build():
    nc = bacc.Bacc("TRN2", target_bir_lowering=False, debug=False, num_devices=8)

    xs = nc.dram_tensor("xs", [C, NSH], BF16, kind="ExternalInput").ap()
    wcat = nc.dram_tensor("wcat", [C, WID], BF16, kind="ExternalInput").ap()
    biaskv = nc.dram_tensor("biaskv", [WID], F32, kind="ExternalInput").ap()
    bq_in = nc.dram_tensor("bq", [CQK, 1], F32, kind="ExternalInput").ap()
    bvg = nc.dram_tensor("bvg", [C], F32, kind="ExternalInput").ap()
    out = nc.dram_tensor("out", [NSH, C], F32, kind="ExternalOutput").ap()

    with tile.TileContext(nc) as tc, ExitStack() as ctx:
        _body(ctx, tc, nc, xs, wcat, biaskv, bq_in, bvg, out)

    nc.compile()
    return nc


def _body(ctx, tc, nc, xs, wcat, biaskv, bq_in, bvg, out):
    singles = ctx.enter_context(tc.tile_pool(name="singles", bufs=1))
    xpool = ctx.enter_context(tc.tile_pool(name="x", bufs=NT512))
    kvpool = ctx.enter_context(tc.tile_pool(name="kv", bufs=2 * GRP))
    khpool = ctx.enter_context(tc.tile_pool(name="kh", bufs=4))
    smalls = ctx.enter_context(tc.tile_pool(name="smalls", bufs=4))
    scpool = ctx.enter_context(tc.tile_pool(name="scratch", bufs=4))
    outpool = ctx.enter_context(tc.tile_pool(name="outp", bufs=3))

    ps_sh = ctx.enter_context(tc.tile_pool(name="ps_sh", bufs=4, space="PSUM"))
    ps_kqv = ctx.enter_context(tc.tile_pool(name="ps_kqv", bufs=3, space="PSUM"))
    ps_f = ctx.enter_context(tc.tile_pool(name="ps_f", bufs=1, space="PSUM"))
    dram = ctx.enter_context(tc.tile_pool(name="dram", bufs=1, space="DRAM"))

    # ---- one-time setup ----
    wcat_sb = singles.tile([128, 2, WID], BF16)
    nc.sync.dma_start(wcat_sb[:], wcat.rearrange("(cb cp) w -> cp cb w", cb=2))
    biaskv_rep = singles.tile([128, WID], F32)  # [bq | bk | zeros(C)]
    nc.gpsimd.dma_start(
        biaskv_rep[:], biaskv.unsqueeze(0).partition_broadcast(128).squeeze(1)
    )
    bq_col = singles.tile([CQK, 1], F32)
    nc.gpsimd.dma_start(bq_col[:], bq_in)
    bv_rep = singles.tile([CQK + 1, C], F32)
    nc.gpsimd.dma_start(
        bv_rep[:], bvg.unsqueeze(0).partition_broadcast(CQK + 1).squeeze(1)
    )
    ident = singles.tile([128, 128], F32)
    make_identity(nc, ident[:])
    ones2 = singles.tile([128, 2], F32)
    nc.vector.memset(ones2[:], 1.0)

    qx = singles.tile([CQK + 1, NSH], BF16)         # layout-A Q rows + ||Q|| row
    ssq_stack = singles.tile([128, NT128], F32)     # sum((Q+bq)^2), col t
    ssk_stack = singles.tile([128, NT128], F32)     # sum((K+bk)^2), col t
    rnormk_stack = singles.tile([128, NT128], F32)
    psf = ps_f.tile([CQK + 1, FD], F32)             # factor accumulator

    kvres_tiles = [None] * NT128
    xt_tiles = [None] * NT512
    pending_tail = None

    def emit_tail(g0):
        normk_g = smalls.tile([128, GRP], F32)
        nc.scalar.sqrt(normk_g[:], ssk_stack[:, g0 : g0 + GRP])
        nc.vector.reciprocal(rnormk_stack[:, g0 : g0 + GRP], normk_g[:])
        for tt in range(g0, g0 + GRP):
            kvt = kvres_tiles[tt]
            kh = khpool.tile([128, CQK + 1], BF16)
            if tt % 2 == 0:
                nc.vector.tensor_scalar_mul(
                    kh[:, 0:CQK], kvt[:, CQK : 2 * CQK], rnormk_stack[:, tt : tt + 1]
                )
            else:
                nc.scalar.mul(
                    kh[:, 0:CQK], kvt[:, CQK : 2 * CQK],
                    rnormk_stack[:, tt : tt + 1],
                )
            nc.gpsimd.tensor_copy(kh[:, CQK : CQK + 1], ones2[:, 0:1])
            nc.tensor.matmul(
                psf[:], kh[:], kvt[:, 2 * CQK : KVW],
                start=(tt == 0), stop=(tt == NT128 - 1),
            )

    # ---- phase 1 ----
    for j in range(NT512):
        xt = xpool.tile([128, 2, 512], BF16)
        nc.sync.dma_start(
            xt[:],
            xs.rearrange("(cb cp) n -> cp cb n", cb=2)[:, :, j * 512 : (j + 1) * 512],
        )

        xt_tiles[j] = xt

        for u in range(4):
            t = j * 4 + u
            if u == 2 and j % 2 == 0 and pending_tail is not None:
                emit_tail(pending_tail)
                pending_tail = None
            # fused [Q^T | K^T | V^T] projection, n-major: [128, 320]
            pskqv = ps_kqv.tile([128, WID], F32)
            for cb in range(2):
                nc.tensor.matmul(
                    pskqv[:], xt[:, cb, u * 128 : (u + 1) * 128], wcat_sb[:, cb, :],
                    start=(cb == 0), stop=(cb == 1),
                )
            # kvres = [Q+bq | K+bk | V | (junk -> ones)]
            kv = kvpool.tile([128, KVW], BF16)
            kvres_tiles[t] = kv
            nc.vector.tensor_tensor(
                kv[:, 0:WID], pskqv[:], biaskv_rep[:], ALU.add
            )
            nc.gpsimd.tensor_copy(kv[:, WID:KVW], ones2[:])
            scr_q = scpool.tile([128, CQK], BF16)
            scr_k = scpool.tile([128, CQK], BF16)
            if t % 2 == 0:
                nc.scalar.activation(
                    scr_q[:], kv[:, 0:CQK], ACTF.Square,
                    accum_out=ssq_stack[:, t : t + 1],
                )
                nc.vector.scalar_tensor_tensor(
                    scr_k[:], kv[:, CQK : 2 * CQK], 1.0, kv[:, CQK : 2 * CQK],
                    ALU.mult, ALU.mult, accum_out=ssk_stack[:, t : t + 1],
                )
            else:
                nc.vector.scalar_tensor_tensor(
                    scr_q[:], kv[:, 0:CQK], 1.0, kv[:, 0:CQK],
                    ALU.mult, ALU.mult, accum_out=ssq_stack[:, t : t + 1],
                )
                nc.scalar.activation(
                    scr_k[:], kv[:, CQK : 2 * CQK], ACTF.Square,
                    accum_out=ssk_stack[:, t : t + 1],
                )

        # ---- group tail (deferred): batched K-norms + factor matmuls ----
        if (j + 1) % (GRP // 4) == 0:
            pending_tail = (j + 1) * 4 - GRP
    if pending_tail is not None:
        emit_tail(pending_tail)
        pending_tail = None

    # ---- phase 1.5: ||Q|| row + AllReduce of factors ----
    normq_stack = singles.tile([128, NT128], F32)
    nc.scalar.sqrt(normq_stack[:], ssq_stack[:])
    pst = ps_sh.tile([NT128, 128], F32, tag="shared")
    nc.tensor.transpose(pst[:], normq_stack[:], ident[:])
    trT = singles.tile([NT128, 128], BF16)
    nc.vector.tensor_copy(trT[:], pst[:])
    row_scratch = dram.tile([NT128, 128], BF16)
    nc.sync.dma_start(row_scratch[:], trT[:])
    nc.sync.dma_start(
        qx[CQK : CQK + 1, :],
        row_scratch[:].rearrange("a b -> (a b)").unsqueeze(0),
    )

    fac_loc = singles.tile([CQK + 1, FD], F32)
    nc.vector.tensor_copy(fac_loc[:], psf[:])
    cc_in = dram.tile([CQK + 1, FD], F32)
    cc_out = dram.tile([2 * (CQK + 1), FD], F32)
    nc.sync.dma_start(cc_in[:], fac_loc[:])
    nc.gpsimd.collective_compute(
        "AllGather",
        ALU.bypass,
        replica_groups=[[0, 1], [2, 3], [4, 5], [6, 7]],
        ins=[cc_in.opt()],
        outs=[cc_out.opt()],
    )
    fac2 = singles.tile([CQK + 1, 2, FD], F32)
    nc.sync.dma_start(fac2[:], cc_out[:].rearrange("(r p) f -> p r f", r=2))
    # ---- gap work: layout-A Q tiles + qx rows (only needed by phase 2) ----
    for j in range(NT512):
        psq = ps_sh.tile([CQK, 512], F32, tag="shared")
        for cb in range(2):
            nc.tensor.matmul(
                psq[:], wcat_sb[:, cb, 0:CQK], xt_tiles[j][:, cb, :],
                start=(cb == 0), stop=(cb == 1),
            )
        nc.scalar.activation(
            qx[0:CQK, j * 512 : (j + 1) * 512], psq[:],
            ACTF.Identity, bias=bq_col[:], scale=1.0,
        )

    # PE warm-keeper (independent of the collective): DVE delay ladder with a
    # dummy matmul after each rung so HAM stays at full clock across the gap.
    warm_a = singles.tile([128, 4096], F32)
    warm_b = singles.tile([128, 4096], F32)
    nc.vector.memset(warm_a[:], 1.0)
    for w in range(8):
        src_t, dst_t = (warm_a, warm_b) if w % 2 == 0 else (warm_b, warm_a)
        nc.vector.tensor_copy(dst_t[:], src_t[:])
        pw = ps_kqv.tile([128, 256], F32, tag="pskqv")
        nc.tensor.matmul(
            pw[:], dst_t[:, 0:128], dst_t[:, 0:256], start=True, stop=True
        )

    facg = singles.tile([CQK + 1, FD], F32)
    nc.vector.tensor_tensor(facg[:], fac2[:, 0, :], fac2[:, 1, :], ALU.add)

    # ---- build Mx [33, 258]:
    #   rows 0-31, cols 0-255: matrix' = facg + Ksum (x) bv'
    #   row 32,    cols 0-255: value_sum' = facg_row32 + N * bv'
    #   col 256:   [Ksum + eps ; N]  (denominator column); col 257 pad
    mx = singles.tile([CQK + 1, FD], BF16)
    tmp32 = singles.tile([CQK, C], F32)
    nc.vector.tensor_scalar_mul(tmp32[:], bv_rep[0:CQK, :], facg[0:CQK, C : C + 1])
    nc.vector.tensor_tensor(mx[0:CQK, 0:C], tmp32[:], facg[0:CQK, 0:C], ALU.add)
    nc.vector.scalar_tensor_tensor(
        mx[CQK : CQK + 1, 0:C], bv_rep[CQK : CQK + 1, :], float(N),
        facg[CQK : CQK + 1, 0:C],
        ALU.mult, ALU.add,
    )
    nc.vector.tensor_scalar_add(mx[0 : CQK + 1, C:FD], facg[0 : CQK + 1, C:FD], EPS)

    # ---- phase 2 ----
    out4 = out.rearrange("(t4 u p) c -> t4 p u c", u=4, p=128)
    for t4 in range(NT128 // 4):
        ot = outpool.tile([128, 4, C], F32)
        for u in range(4):
            t = t4 * 4 + u
            ps2 = ps_sh.tile([128, FD], F32, tag="shared")
            nc.tensor.matmul(
                ps2[:], qx[:, t * 128 : (t + 1) * 128], mx[:], start=True, stop=True
            )
            s_col = smalls.tile([128, 1], F32)
            nc.vector.reciprocal(s_col[:], ps2[:, C : C + 1])
            if t % 2 == 0:
                nc.vector.tensor_scalar_mul(ot[:, u, :], ps2[:, 0:C], s_col[:])
            else:
                nc.scalar.mul(ot[:, u, :], ps2[:, 0:C], s_col[:])
        nc.sync.dma_start(out4[t4], ot[:])


def _get_nc():
    if "nc" not in _CACHE:
        _CACHE["nc"] = _build()
    return _CACHE["nc"]


def _prep_in_maps(x, Wq, bq, Wk, bk, Wv, bv, gamma):
    g = float(np.asarray(gamma).reshape(-1)[0])
    wcat = np.concatenate(
        [
            Wq.T.astype(np.float32),
            Wk.T.astype(np.float32),
            (g * Wv).T.astype(np.float32),
        ],
        axis=1,
    ).astype(ml_dtypes.bfloat16)
    wcat = np.ascontiguousarray(wcat)
    biaskv = np.concatenate(
        [bq.astype(np.float32), bk.astype(np.float32), np.zeros(C, np.float32)]
    )
    bvg = np.ascontiguousarray(g * bv, dtype=np.float32)
    bq_col = np.ascontiguousarray(bq.reshape(CQK, 1), dtype=np.float32)

    xf = np.asarray(x, dtype=np.float32).reshape(B, C, N)
    in_maps = []
    for core in range(8):
        b, h = core // 2, core % 2
        xsh = np.ascontiguousarray(
            xf[b, :, h * NSH : (h + 1) * NSH].astype(ml_dtypes.bfloat16)
        )
        in_maps.append(
            {
                "xs": xsh,
                "wcat": wcat,
                "biaskv": biaskv,
                "bq": bq_col,
                "bvg": bvg,
            }
        )
    return in_maps


def run(inputs, trace=False):
    nc = _get_nc()
    in_maps = _prep_in_maps(**inputs)
    res = bass_utils.run_bass_kernel_spmd(
        nc, in_maps, core_ids=list(range(8)), trace=trace
    )
    outf = np.empty((B, C, N), np.float32)
    for core in range(8):
        b, h = core // 2, core % 2
        outf[b, :, h * NSH : (h + 1) * NSH] = res.results[core]["out"].T
    return outf.reshape(B, C, HH, WW), res


def kernel(**inputs):
    out, _ = run(inputs, trace=False)
    return out

